# revision 1
# baseline (speedup 1.0000x reference)
"""CTC loss (keras ctc_batch_cost semantics) on 8 Trainium2 NeuronCores.

Algorithm: linear-space CTC forward DP, reformulated as a *wavefront* over
extended-label lanes.  For each label lane k the whole time axis is computed
with one hardware linear-recurrence instruction (tensor_tensor_scan on the
DVE), so the serial chain is over k (129 steps), not over t (512 steps).

  E[k]_t = pb_t * (E[k]_{t-1} + O[k-1]_{t-1})                 (blank state 2k)
  O[k]_t = pl[k]_t * (O[k]_{t-1} + E[k]_{t-1} + kap_k*O[k-1]_{t-1})  (label 2k+1)

Probabilities are pre-scaled by 1/r_t with r_t = sum_s p_s^2 / sum_s p_s
(self-weighted mean over extended states) so the linear-space values stay
inside fp32 range for all 512 steps; the loss adds back sum_t log r_t.

The per-(b,t) gather y_pred[b,t,y_true[b,k]] is done as a one-hot matmul on
the tensor engine; the [k,t]->[b,t] re-layout is a flat SBUF->SBUF DMA.
Batch is sharded 32 per core (pure data parallelism).
"""

import sys

for _p in ("/opt/trn_rl_repo",):
    if _p not in sys.path:
        sys.path.insert(0, _p)

from contextlib import ExitStack

import numpy as np

import concourse.bacc as bacc
import concourse.bass as bass
import concourse.tile as tile
from concourse import mybir
from concourse.bass_utils import run_bass_kernel_spmd

F32 = mybir.dt.float32
AF = mybir.ActivationFunctionType
OP = mybir.AluOpType

B, T, C, L = 256, 512, 256, 128
NCORES = 8
BS = B // NCORES
EPS = 1e-7
BLANK = C - 1

_nc_cache = {}


def build_nc(bs=BS, t=T, c=C, l=L):
    key = (bs, t, c, l)
    if key in _nc_cache:
        return _nc_cache[key]
    CT = c // 128
    GRP = min(8, bs)
    nc = bacc.Bacc("TRN2")
    ypT = nc.declare_dram_parameter("ypT", [bs, c, t], F32, isOutput=False)
    Gd = nc.declare_dram_parameter("G", [bs, c, l], F32, isOutput=False)
    cntd = nc.declare_dram_parameter("cnt", [bs, c, 1], F32, isOutput=False)
    kapd = nc.declare_dram_parameter("kap", [bs, l], F32, isOutput=False)
    lossd = nc.declare_dram_parameter("loss", [bs, 1], F32, isOutput=True)

    with ExitStack() as ctx:
        tc = ctx.enter_context(tile.TileContext(nc))
        pers = ctx.enter_context(tc.tile_pool(name="pers", bufs=1))
        ypool = ctx.enter_context(tc.tile_pool(name="y", bufs=2))
        gpool = ctx.enter_context(tc.tile_pool(name="g", bufs=2))
        y2pool = ctx.enter_context(tc.tile_pool(name="y2", bufs=3))
        bcpool = ctx.enter_context(tc.tile_pool(name="bc", bufs=3))
        pspool = ctx.enter_context(
            tc.tile_pool(name="ps", bufs=3, space=bass.MemorySpace.PSUM)
        )
        psspool = ctx.enter_context(
            tc.tile_pool(name="pss", bufs=2, space=bass.MemorySpace.PSUM)
        )
        drampool = ctx.enter_context(
            tc.tile_pool(name="dram", bufs=2, space=bass.MemorySpace.DRAM)
        )

        pl_big = pers.tile([128, bs * t], F32)  # scaled gathered label probs
        PB = pers.tile([bs, t], F32)
        INVR = pers.tile([bs, t], F32)
        PBS = pers.tile([bs, t], F32)
        KAP = pers.tile([bs, l], F32)
        LOGACC = pers.tile([bs, 1], F32)
        SCR = pers.tile([bs, t], F32)
        ZERO = pers.tile([bs, t], F32)
        FIN = pers.tile([bs, 1], F32)
        LLOG = pers.tile([bs, 1], F32)
        LOSS = pers.tile([bs, 1], F32)

        nc.sync.dma_start(KAP[:], kapd[:])
        nc.gpsimd.memset(ZERO[:], 0.0)

        # ---------------- phase A: gather + scaling, in groups of GRP ----
        for g0 in range(0, bs, GRP):
            ng = min(GRP, bs - g0)
            ytiles = {}
            # A1: load y, blank rows, squares, s1/s2 matmuls
            i1 = bcpool.tile([GRP, t], F32, tag="i1")
            iv = bcpool.tile([GRP, t], F32, tag="iv")
            for loc in range(ng):
                b = g0 + loc
                psg = psspool.tile([33, t], F32, tag="psg")
                cts = []
                for ci in range(CT):
                    y = ypool.tile([128, t], F32, tag=f"Y{loc}_{ci}")
                    nc.sync.dma_start(y[:], ypT[b, ci * 128 : (ci + 1) * 128, :])
                    ytiles[(loc, ci)] = y
                    cn = gpool.tile([128, 1], F32, tag=f"cn{ci}")
                    nc.sync.dma_start(cn[:], cntd[b, ci * 128 : (ci + 1) * 128, :])
                    cts.append(cn)
                nc.sync.dma_start(PB[b : b + 1, :], ypT[b, BLANK : BLANK + 1, :])
                for ci in range(CT):
                    y2 = y2pool.tile([128, t], F32, tag="Y2")
                    nc.scalar.activation(y2[:], ytiles[(loc, ci)][:], AF.Square)
                    nc.tensor.matmul(
                        psg[0:1, :],
                        cts[ci][:],
                        ytiles[(loc, ci)][:],
                        start=(ci == 0),
                        stop=(ci == CT - 1),
                    )
                    nc.tensor.matmul(
                        psg[32:33, :],
                        cts[ci][:],
                        y2[:],
                        start=(ci == 0),
                        stop=(ci == CT - 1),
                    )
                # evac s1/s2 rows via SBUF bounce (engines can't start at
                # partition b; DMA can)
                pse = y2pool.tile([33, t], F32, tag="pse")
                nc.scalar.copy(pse[0:1, :], psg[0:1, :])
                nc.scalar.copy(pse[32:33, :], psg[32:33, :])
                nc.sync.dma_start(i1[loc : loc + 1, :], pse[0:1, :])
                nc.sync.dma_start(iv[loc : loc + 1, :], pse[32:33, :])
            # invr = s1 / s2   (r = s2/s1 = selfweighted mean prob)
            nc.vector.reciprocal(iv[0:ng, :], iv[0:ng, :])
            nc.vector.tensor_mul(iv[0:ng, :], iv[0:ng, :], i1[0:ng, :])
            nc.sync.dma_start(INVR[g0 : g0 + ng, :], iv[0:ng, :])
            # A2: gather matmul + scaled evac
            for loc in range(ng):
                b = g0 + loc
                gts = []
                for ci in range(CT):
                    gt = gpool.tile([128, l], F32, tag=f"G{ci}")
                    nc.sync.dma_start(gt[:], Gd[b, ci * 128 : (ci + 1) * 128, :])
                    gts.append(gt)
                ps = pspool.tile([128, t], F32, tag="plps")
                for ci in range(CT):
                    nc.tensor.matmul(
                        ps[0:l, :],
                        gts[ci][:],
                        ytiles[(loc, ci)][:],
                        start=(ci == 0),
                        stop=(ci == CT - 1),
                    )
                ivd = drampool.tile([1, t], F32, tag="ivd")
                nc.sync.dma_start(ivd[:], iv[loc : loc + 1, :])
                bc = bcpool.tile([128, t], F32, tag="bc")
                nc.sync.dma_start(bc[:], ivd[:].to_broadcast((128, t)))
                # pl_big[0:l, b] = (ps + EPS) * invr_bcast
                nc.vector.scalar_tensor_tensor(
                    pl_big[0:l, b * t : (b + 1) * t],
                    ps[0:l, :],
                    float(EPS),
                    bc[0:l, :],
                    OP.add,
                    OP.mult,
                )
        # scaled blank probs + log-accumulator
        nc.vector.scalar_tensor_tensor(
            PBS[:], PB[:], float(EPS), INVR[:], OP.add, OP.mult
        )
        nc.scalar.activation(SCR[:], INVR[:], AF.Ln, accum_out=LOGACC[:])

        # ---------------- phase B+C: wavefront over label lanes ----------
        E0 = pers.tile([bs, 1 + t], F32)
        Ebuf = [pers.tile([bs, 1 + t], F32, name=f"Eb{i}") for i in range(2)]
        Obuf = [pers.tile([bs, 1 + t], F32, name=f"Ob{i}") for i in range(3)]
        Dbuf = [pers.tile([bs, t], F32, name=f"Db{i}") for i in range(2)]
        plbuf = [pers.tile([bs, t], F32, name=f"plb{i}") for i in range(4)]
        nc.gpsimd.memset(E0[:, 0:1], 1.0)
        for tb in Ebuf + Obuf:
            nc.gpsimd.memset(tb[:, 0:1], 0.0)

        def shuffle(k, dst):
            # row k of pl_big, b-blocks -> [bs, t]
            nc.sync.dma_start(dst[:], pl_big[k : k + 1, :])

        # k = 0
        nc.vector.tensor_tensor_scan(
            E0[:, 1 : 1 + t], ZERO[:], PBS[:], E0[:, 0:1], OP.add, OP.mult
        )
        shuffle(0, plbuf[0])
        nc.vector.tensor_tensor_scan(
            Obuf[0][:, 1 : 1 + t],
            E0[:, 0:t],
            plbuf[0][:],
            Obuf[0][:, 0:1],
            OP.add,
            OP.mult,
        )
        prevO = Obuf[0]
        for k in range(1, l):
            Ek = Ebuf[k % 2]
            Ok = Obuf[k % 3]
            dl = Dbuf[k % 2]
            plk = plbuf[k % 4]
            shuffle(k, plk)
            nc.vector.tensor_tensor_scan(
                Ek[:, 1 : 1 + t], prevO[:, 0:t], PBS[:], Ek[:, 0:1], OP.add, OP.mult
            )
            nc.vector.scalar_tensor_tensor(
                dl[:], prevO[:, 0:t], KAP[:, k : k + 1], Ek[:, 0:t], OP.mult, OP.add
            )
            nc.vector.tensor_tensor_scan(
                Ok[:, 1 : 1 + t], dl[:], plk[:], Ok[:, 0:1], OP.add, OP.mult
            )
            prevO = Ok
        EL = Ebuf[l % 2]
        nc.vector.tensor_tensor_scan(
            EL[:, 1 : 1 + t], prevO[:, 0:t], PBS[:], EL[:, 0:1], OP.add, OP.mult
        )
        nc.vector.tensor_add(FIN[:], EL[:, t : t + 1], prevO[:, t : t + 1])
        nc.scalar.activation(LLOG[:], FIN[:], AF.Ln)
        nc.vector.tensor_sub(LOSS[:], LOGACC[:], LLOG[:])
        nc.sync.dma_start(lossd[:], LOSS[:])

    nc.finalize()
    _nc_cache[key] = nc
    return nc


def host_prep(y_true, y_pred, bs=BS, t=T, c=C, l=L):
    """Per-core input maps: transposed probs, one-hot gather matrix, counts,
    skip mask."""
    ncores = y_true.shape[0] // bs
    maps = []
    for core in range(ncores):
        sl = slice(core * bs, (core + 1) * bs)
        yt = np.asarray(y_true[sl], dtype=np.int32)
        ypT = np.ascontiguousarray(
            np.asarray(y_pred[sl], dtype=np.float32).transpose(0, 2, 1)
        )
        G = (yt[:, None, :] == np.arange(c, dtype=np.int32)[None, :, None]).astype(
            np.float32
        )
        cnt = G.sum(axis=2, keepdims=True)
        cnt[:, c - 1, 0] = l + 1.0  # blank multiplicity in extended states
        kap = np.zeros((bs, l), dtype=np.float32)
        kap[:, 1:] = (yt[:, 1:] != yt[:, :-1]).astype(np.float32)
        maps.append({"ypT": ypT, "G": G, "cnt": cnt, "kap": kap})
    return maps


def kernel(y_true, y_pred):
    nc = build_nc()
    maps = host_prep(y_true, y_pred)
    res = run_bass_kernel_spmd(nc, maps, list(range(NCORES)))
    loss = np.concatenate([res.results[i]["loss"] for i in range(NCORES)], axis=0)
    return loss.astype(np.float32)



# revision 22
# speedup vs baseline: 1.4691x; 1.4691x over previous
"""CTC loss (keras ctc_batch_cost semantics) on 8 Trainium2 NeuronCores.

Linear-space CTC forward DP as a *blocked wavefront* over (label lane k,
time chunk j).  T=512 is split into J=4 chunks of Tc=128; block (k, j)
depends on (k-1, j) and (k, j-1), so anti-diagonal d = k + j processes 4
blocks at once, stacked into all 128 partitions (32 batch x 4 chunks,
partition p = 32*j + b).  Each step runs three short DVE ops
([128, 128] free dim instead of [32, 512]):

  E[k]_t = pb_t * (E[k]_{t-1} + O[k-1]_{t-1})                 (blank 2k)
  dl_t   = kap_k * O[k-1]_t + E[k]_t
  O[k]_t = pl[k]_t * (O[k]_{t-1} + dl_{t-1})                  (label 2k+1)

Probabilities are pre-scaled by 1/r_t (r_t = sum cnt*p^2 / sum cnt*p over
the upper 128 classes incl. blank) so linear-space values stay in fp32
range; the loss adds back sum_t log r_t.  Any consistent r works, so the
half-class estimate is exact, not an approximation.

The per-(b,t) gather y_pred[b,t,y_true[b,k]] is a one-hot matmul in bf16;
s1/s2 rows for all b land in one [32, T] PSUM tile via count-vectors
one-hot-placed by batch column.  Lane data is staged once into a skewed
[128, 132*128] SBUF layout (PLS) so the 132-step wavefront issues no DMAs.
Batch is sharded 32 per core (pure data parallelism).
"""

import sys

for _p in ("/opt/trn_rl_repo",):
    if _p not in sys.path:
        sys.path.insert(0, _p)

from contextlib import ExitStack

import numpy as np
import ml_dtypes

import concourse.bacc as bacc
import concourse.bass as bass
import concourse.tile as tile
from concourse import mybir
from concourse.bass_utils import run_bass_kernel_spmd

F32 = mybir.dt.float32
BF16 = mybir.dt.bfloat16
AF = mybir.ActivationFunctionType
OP = mybir.AluOpType

B, T, C, L = 256, 512, 256, 128
NCORES = 8
BS = B // NCORES          # 32 batch per core
TC = 128                  # time chunk
J = 4                     # chunks (J*TC == T)
NSTEP = L + J             # 132 wavefront steps (k = 0..128 incl final blank lane)
NG = 8                    # batch group size for phase A pipelining
NGRP = BS // NG
EPS = 1e-7
BLANK = C - 1

_nc_cache = {}


def build_nc(bs=BS, t=T, c=C, l=L):
    key = (bs, t, c, l)
    if key in _nc_cache:
        return _nc_cache[key]
    nc = bacc.Bacc("TRN2")
    ypT = nc.declare_dram_parameter("ypT", [bs, c, t], BF16, isOutput=False)
    Gd = nc.declare_dram_parameter("G", [bs, 2, 128, l], BF16, isOutput=False)
    cntd = nc.declare_dram_parameter("cnt", [bs, 2, 128, bs], BF16, isOutput=False)
    kapd = nc.declare_dram_parameter("kap", [128, NSTEP], F32, isOutput=False)
    shiftd = nc.declare_dram_parameter("shift", [128, 128], BF16, isOutput=False)
    lossd = nc.declare_dram_parameter("loss", [bs, 1], F32, isOutput=True)

    with ExitStack() as ctx:
        tc = ctx.enter_context(tile.TileContext(nc))
        pers = ctx.enter_context(tc.tile_pool(name="pers", bufs=1))
        ypool = ctx.enter_context(tc.tile_pool(name="y", bufs=2))
        gpool = ctx.enter_context(tc.tile_pool(name="g", bufs=2))
        y2pool = ctx.enter_context(tc.tile_pool(name="y2", bufs=3))
        bcpool = ctx.enter_context(tc.tile_pool(name="bc", bufs=2))
        pltpool = ctx.enter_context(tc.tile_pool(name="plt", bufs=2))
        tmppool = ctx.enter_context(tc.tile_pool(name="tmp", bufs=2))
        dlpool = ctx.enter_context(tc.tile_pool(name="dl", bufs=2))
        s1pool = ctx.enter_context(
            tc.tile_pool(name="s1", bufs=2, space=bass.MemorySpace.PSUM)
        )
        s2pool = ctx.enter_context(
            tc.tile_pool(name="s2", bufs=2, space=bass.MemorySpace.PSUM)
        )
        pspool = ctx.enter_context(
            tc.tile_pool(name="ps", bufs=2, space=bass.MemorySpace.PSUM)
        )
        carpool = ctx.enter_context(
            tc.tile_pool(name="car", bufs=2, space=bass.MemorySpace.PSUM)
        )
        drampool = ctx.enter_context(
            tc.tile_pool(name="dram", bufs=1, space=bass.MemorySpace.DRAM)
        )

        # ---------------- persistent tiles ------------------------------
        PLS = pers.tile([128, NSTEP, TC], BF16)   # staged lane probs, skewed by j
        PBS_PAR = pers.tile([128, TC], BF16)      # scaled blank probs, chunk layout
        KAPS = pers.tile([128, NSTEP], F32)       # skip flags, skewed by j
        PB = pers.tile([bs, t], BF16)             # blank prob rows
        PBSrow = pers.tile([bs, t], BF16)
        INVR = pers.tile([bs, t], BF16)           # 1/r rows (bf16, used everywhere)
        SCR = pers.tile([bs, t], BF16)
        LOGACC = pers.tile([bs, 1], F32)
        FIN = pers.tile([128, 1], F32)
        LL = pers.tile([128, 1], F32)
        LLmv = pers.tile([bs, 1], F32)
        LOSS = pers.tile([bs, 1], F32)
        Ebuf = [pers.tile([128, 1 + TC], BF16, name=f"Eb{i}") for i in range(2)]
        Obuf = [pers.tile([128, 1 + TC], BF16, name=f"Ob{i}") for i in range(2)]
        DRT = drampool.tile([NGRP, NG * t], BF16)
        # DRAM bounce for lane staging: SBUF APs cannot lead with a free dim,
        # so the (k,b,u)->(b,k,u) reorder happens via DRAM strides instead
        PLTD = drampool.tile([NGRP, 128, NG, J, TC], BF16)

        nc.sync.dma_start(KAPS[:], kapd[:])
        # per-source-b one-hot count stationaries: cnt2[:, b, ci, :] has only
        # column b nonzero, so matmul(cnt2[:,b,ci,:], y_b) lands in PSUM row b
        cnt2 = pers.tile([128, bs, 2, bs], BF16)
        nc.sync.dma_start(cnt2[:], cntd[:].rearrange("b ci p col -> p b ci col"))
        SHIFT = pers.tile([128, 128], BF16)  # one-hot: out[p] = in[p-32], rows 0:32 = 0
        nc.sync.dma_start(SHIFT[:], shiftd[:])
        # blank prob rows for every b in one strided DMA
        nc.sync.dma_start(PB[:], ypT[:, BLANK, :])

        # zero init: O tiles fully; E carry cols; PLS skew edges
        for tb in Obuf + Ebuf:
            nc.gpsimd.memset(tb[:], 0.0)
        nc.gpsimd.memset(Ebuf[0][0:32, 0:1], 1.0)  # E[0]_{-1} = 1 (lane 0, chunk 0)
        for j in range(J):
            if j > 0:
                nc.gpsimd.memset(PLS[32 * j : 32 * (j + 1), 0:j, :], 0.0)
            nc.gpsimd.memset(PLS[32 * j : 32 * (j + 1), j + l : NSTEP, :], 0.0)

        # ---------------- phase A: gather + scaling ----------------------
        for g in range(NGRP):
            yg = ypool.tile([128, NG, 2, t], BF16, tag="yg")
            gg = gpool.tile([128, NG, 2, l], BF16, tag="gg")
            nc.sync.dma_start(yg[:], ypT[g * NG : (g + 1) * NG, :, :].rearrange(
                "b (ci p) t -> p b ci t", ci=2, p=128))
            nc.scalar.dma_start(gg[:], Gd[g * NG : (g + 1) * NG, :, :, :].rearrange(
                "b ci p k -> p b ci k"))

            # s1/s2 for the group's 8 b's into rows 8g..8g+7 of [32, t] PSUM
            S1g = s1pool.tile([bs, t], F32, tag="S1g")
            S2g = s2pool.tile([bs, t], F32, tag="S2g")
            for bp in range(NG):
                b = g * NG + bp
                for ci in range(2):
                    yci = yg[:, bp, ci, :]
                    y2 = y2pool.tile([128, t], BF16, tag="y2")
                    nc.vector.tensor_mul(y2[:], yci, yci)
                    st = bp == 0 and ci == 0
                    sp = bp == NG - 1 and ci == 1
                    nc.tensor.matmul(S1g[:], cnt2[:, b, ci, :], yci, start=st, stop=sp)
                    nc.tensor.matmul(S2g[:], cnt2[:, b, ci, :], y2[:], start=st, stop=sp)
            # invr rows for this group (other rows are 0/0 junk, never read)
            TMPa = tmppool.tile([bs, t], F32, tag="TMPa")
            TMPb = tmppool.tile([bs, t], BF16, tag="TMPb")
            nc.vector.reciprocal(TMPa[:], S2g[:])
            nc.vector.tensor_mul(TMPb[:], TMPa[:], S1g[:])
            nc.sync.dma_start(INVR[g * NG : (g + 1) * NG, :], TMPb[g * NG : (g + 1) * NG, :])
            nc.sync.dma_start(DRT[g : g + 1, :], TMPb[g * NG : (g + 1) * NG, :])
            # broadcast invr rows to all 128 partitions for the gather evac
            bcg = bcpool.tile([128, NG * t], BF16, tag="bcg")
            nc.scalar.dma_start(bcg[:], DRT[g : g + 1, :].to_broadcast((128, NG * t)))

            # gather matmuls + scaled evac into PLTg
            pltg = pltpool.tile([128, NG, J, TC], BF16, tag="pltg")
            for bp in range(NG):
                psg = pspool.tile([128, t], F32, tag="psg")
                for ci in range(2):
                    nc.tensor.matmul(
                        psg[0:l, :], gg[:, bp, ci, :], yg[:, bp, ci, :],
                        start=(ci == 0), stop=(ci == 1),
                    )
                nc.vector.scalar_tensor_tensor(
                    pltg[:, bp, :, :].rearrange("p j u -> p (j u)"),
                    psg[0:l, :], float(EPS),
                    bcg[:, bp * t : (bp + 1) * t],
                    OP.add, OP.mult,
                )
            # stage into the skewed wavefront layout: PLS[32j+8g+b', k+j, u]
            nc.sync.dma_start(PLTD[g], pltg[:])
            for j in range(J):
                nc.scalar.dma_start(
                    PLS[32 * j + NG * g : 32 * j + NG * (g + 1), j : j + l, :],
                    PLTD[g, :, :, j, :].rearrange("k b u -> b k u"),
                )

        # scaled blank probs -> chunk-parallel layout
        nc.vector.scalar_tensor_tensor(PBSrow[:], PB[:], float(EPS), INVR[:], OP.add, OP.mult)
        for j in range(J):
            nc.sync.dma_start(
                PBS_PAR[32 * j : 32 * (j + 1), :],
                PBSrow[:, j * TC : (j + 1) * TC],
            )
        # log-accumulator: sum_t ln(1/r_t)
        nc.scalar.activation(SCR[:], INVR[:], AF.Ln, accum_out=LOGACC[:])

        # ---------------- phase B: blocked wavefront ---------------------
        # Carries (end-of-chunk values) must shift down one 32-partition j
        # block; engines can't do partition-shifted APs, so the shift runs
        # on the otherwise-idle PE via a one-hot shift matrix.  Scans read
        # their init directly from the PSUM carry; a full-width ACT copy
        # fills col 0 of the tile for the dl stt's t-1 stream.
        for d in range(NSTEP):
            Ec, Ep = Ebuf[d % 2], Ebuf[1 - d % 2]
            Oc, Op_ = Obuf[d % 2], Obuf[1 - d % 2]
            if d == 0:
                einit = Ec[:, 0:1]  # preset: 1 in rows 0:32 (E[0] seed), else 0
                oinit = Oc[:, 0:1]  # preset zeros
            else:
                car = carpool.tile([128, 2], F32, tag="car")
                nc.tensor.matmul(car[:, 0:1], SHIFT[:], Ep[:, TC : TC + 1], start=True, stop=True)
                nc.tensor.matmul(car[:, 1:2], SHIFT[:], Op_[:, TC : TC + 1], start=True, stop=True)
                nc.scalar.copy(Ec[:, 0:1], car[:, 0:1])
                nc.scalar.copy(Oc[:, 0:1], car[:, 1:2])
                einit, oinit = car[:, 0:1], car[:, 1:2]
            nc.vector.tensor_tensor_scan(
                Ec[:, 1 : 1 + TC], Op_[:, 0:TC], PBS_PAR[:], einit, OP.add, OP.mult
            )
            dl = dlpool.tile([128, TC], BF16, tag="dl")
            nc.vector.scalar_tensor_tensor(
                dl[:], Op_[:, 0:TC], KAPS[:, d : d + 1], Ec[:, 0:TC], OP.mult, OP.add
            )
            nc.vector.tensor_tensor_scan(
                Oc[:, 1 : 1 + TC], dl[:], PLS[:, d, :], oinit, OP.add, OP.mult
            )

        # ---------------- loss -------------------------------------------
        ELast = Ebuf[(NSTEP - 1) % 2]   # E[128] chunk 3 (step 131)
        OLast = Obuf[(NSTEP - 2) % 2]   # O[127] chunk 3 (step 130)
        nc.vector.tensor_add(FIN[96:128, :], ELast[96:128, TC : TC + 1], OLast[96:128, TC : TC + 1])
        nc.scalar.activation(LL[96:128, :], FIN[96:128, :], AF.Ln)
        nc.sync.dma_start(LLmv[:], LL[96:128, :])
        nc.vector.tensor_sub(LOSS[:], LOGACC[:], LLmv[:])
        nc.sync.dma_start(lossd[:], LOSS[:])

    nc.finalize()
    _nc_cache[key] = nc
    return nc


def host_prep(y_true, y_pred, bs=BS, t=T, c=C, l=L):
    """Per-core input maps: transposed bf16 probs, one-hot gather matrix,
    batch-placed count columns, skewed skip flags."""
    ncores = y_true.shape[0] // bs
    maps = []
    for core in range(ncores):
        sl = slice(core * bs, (core + 1) * bs)
        yt = np.asarray(y_true[sl], dtype=np.int32)
        ypT = np.ascontiguousarray(
            np.asarray(y_pred[sl], dtype=np.float32).transpose(0, 2, 1)
        ).astype(ml_dtypes.bfloat16)
        G = (yt[:, None, :] == np.arange(c, dtype=np.int32)[None, :, None]).astype(
            ml_dtypes.bfloat16
        ).reshape(bs, 2, 128, l)
        # extended-state counts, one-hot placed: cnt[b, ci, c, col] nonzero
        # only at col == b so each b's matmul lands in its own PSUM row
        cnt = np.zeros((bs, 2, 128, bs), dtype=np.float32)
        for b in range(bs):
            idx, n = np.unique(yt[b], return_counts=True)
            cnt[b, idx // 128, idx % 128, b] = n
            cnt[b, 1, 127, b] = l + 1.0
        kap = np.zeros((bs, l), dtype=np.float32)
        kap[:, 1:] = (yt[:, 1:] != yt[:, :-1]).astype(np.float32)
        kaps = np.zeros((128, NSTEP), dtype=np.float32)
        for j in range(J):
            kaps[32 * j : 32 * (j + 1), j : j + l] = kap
        shift = np.zeros((128, 128), dtype=ml_dtypes.bfloat16)
        for q in range(96):
            shift[q, q + 32] = 1.0
        maps.append({
            "ypT": ypT,
            "G": G,
            "cnt": cnt.astype(ml_dtypes.bfloat16),
            "kap": kaps,
            "shift": shift,
        })
    return maps


def kernel(y_true, y_pred):
    nc = build_nc()
    maps = host_prep(y_true, y_pred)
    res = run_bass_kernel_spmd(nc, maps, list(range(NCORES)))
    loss = np.concatenate([res.results[i]["loss"] for i in range(NCORES)], axis=0)
    return loss.astype(np.float32)
